# revision 45
# baseline (speedup 1.0000x reference)
"""Trainium2 Bass kernel v5 for Llama GQA attention prefill (S=2048).

Tensor-parallel over heads across 8 NeuronCores; each core owns 4 query
heads + 1 KV head, computes its partial o_proj contribution; host sums.

v4/v5 changes vs v3 (459.6us -> ~409us mean NTFF exec time):
- Softmax denominator: exp tiles are pair-summed on the DVE (bf16
  2x-rate tensor_adds) and the one-hot dn matmuls stream the pair sums,
  halving the PE's denominator column traffic; pairs flush two-at-a-time
  with the head loop outer so each selector stationary loads once.
  (Quad-level summing was tried and is NEUTRAL: phase 2 is ACT-bound on
  the exp stream at ~2.3us/kt, so further PE relief just idles the PE.)
- Epilogue: reciprocal_approx_fast reading the dn4 PSUM bank directly.
- Phase 1 DMA: w rides the SP queue, x0 the ACT queue (descriptor gen
  ~0.7us/dma_start on one queue cannot keep ahead of the PE's 1.28us/kb
  consumption); consts issued behind the x0 stream; xta prefetches for
  sb2/sb3 issued lazily; cos2/sin2 DMA only 64 partitions and the
  mirrored/negated halves are reconstructed on the idle DVE (0.5MB less
  HBM traffic in the bandwidth-critical first ~50us).

v3 design (unchanged): per-512-block qk/kT/vt/at tensors for
fine-grained cross-phase deps; phase-1 rope lags one projection group;
per-qb accs freed early via unnormalized bf16 copies; qb0 processed
last so its thin-tile stalls border phase 3.
"""

import sys

if "/opt/trn_rl_repo" not in sys.path:
    sys.path.insert(0, "/opt/trn_rl_repo")

import numpy as np
import ml_dtypes

BF = ml_dtypes.bfloat16

S = 2048
HID = 4096
D = 128
H = 32
HKV = 8
NCORES = 8
NQ = H // NCORES  # 4 query heads per core
ROPE_THETA = 10000.0

_NC_CACHE = {}


def build_nc(s=S, hid=HID, nq=NQ):
    import concourse.bass as bass
    import concourse.mybir as mybir
    import concourse.tile as tile
    from concourse import bacc

    f32 = mybir.dt.float32
    bf16 = mybir.dt.bfloat16
    Exp = mybir.ActivationFunctionType.Exp

    KB = hid // 128   # hidden contraction blocks (32)
    SBn = s // 512    # 512-wide sequence blocks (4)
    STn = s // 128    # seq tiles (16)
    NDB = nq + 2      # projection d-blocks: k, v, q0..q{nq-1}
    WC = NDB * 128
    HB = hid // 512   # output hidden blocks (8)

    nc = bacc.Bacc("TRN2")

    xT_d = nc.dram_tensor("xT", [hid, s], bf16, kind="ExternalInput")
    w_d = nc.dram_tensor("wqkvT", [hid, WC], bf16, kind="ExternalInput")
    wo_d = nc.dram_tensor("woT", [nq * 128, hid], bf16, kind="ExternalInput")
    cos2_d = nc.dram_tensor("cos2", [128, s], bf16, kind="ExternalInput")
    sin2_d = nc.dram_tensor("sin2", [128, s], bf16, kind="ExternalInput")
    tri_d = nc.dram_tensor("tri", [128, 128], bf16, kind="ExternalInput")
    swpm_d = nc.dram_tensor("swpm", [128, 128], bf16, kind="ExternalInput")
    onesel_d = nc.dram_tensor("onesel", [128, nq * nq], bf16,
                              kind="ExternalInput")
    selbc_d = nc.dram_tensor("selbc", [nq, nq * 128], bf16,
                             kind="ExternalInput")
    out_d = nc.dram_tensor("out", [s, hid], bf16, kind="ExternalOutput")

    with tile.TileContext(nc) as tc:
        with (
            tc.tile_pool(name="const", bufs=1) as const_pool,
            tc.tile_pool(name="qkv", bufs=1) as qkv_pool,
        ):
            tri = const_pool.tile([128, 128], bf16, tag="tri")
            onesel = const_pool.tile([128, nq * nq], bf16, tag="onesel")
            selbc = const_pool.tile([nq, nq * 128], bf16, tag="selbc")
            swpm = const_pool.tile([128, 128], bf16, tag="swpm")
            cos2 = const_pool.tile([128, s], bf16, tag="cos2")
            sin2 = const_pool.tile([128, s], bf16, tag="sin2")
            # const DMAs are issued after the sb0 x stream (below): they are
            # not needed before the first rope (~52us), and issuing them
            # first would delay the x0 descriptor cadence at startup.

            # per-(head, seq-block) tensors; index nq == kT
            qk = [
                [
                    qkv_pool.tile([128, 512], bf16, tag=f"qk{i}_{sb}",
                                  name=f"qk{i}_{sb}")
                    for sb in range(SBn)
                ]
                for i in range(nq + 1)
            ]
            vts = [
                qkv_pool.tile([128, 4, 128], bf16, tag=f"vt{sb}",
                              name=f"vt{sb}")
                for sb in range(SBn)
            ]

            # ---------------- Phase 1: projections + RoPE ----------------
            with (
                tc.tile_pool(name="xt0", bufs=1) as xt0_pool,
                tc.tile_pool(name="xta", bufs=2) as xta_pool,
                tc.tile_pool(name="wsb", bufs=1) as w_pool,
                tc.tile_pool(name="pp", bufs=6, space="PSUM") as pp,
                tc.tile_pool(name="spp", bufs=2, space="PSUM") as spp,
                tc.tile_pool(name="rtmp", bufs=4) as rt,
                tc.tile_pool(name="vstage", bufs=2) as vs,
            ):
                w_sb = w_pool.tile([128, KB, WC], bf16, tag="wsb")
                wv_view = w_d[:, :].rearrange("(t p) c -> p t c", p=128)
                xv_view = xT_d[:, :].rearrange("(t p) s -> p t s", p=128)

                # sb=0: per-kb x/w DMAs in consumption order. Descriptor
                # generation costs ~0.7us per dma_start; one queue generating
                # both (1.4us/kb) can't keep ahead of the PE's 1.28us/kb
                # consumption, so w rides the SP queue and x the ACT queue.
                xts0 = []
                for kb in range(KB):
                    xtile = xt0_pool.tile([128, 512], bf16, tag=f"x0_{kb}",
                                          name=f"x0_{kb}")
                    nc.sync.dma_start(w_sb[:, kb, :], wv_view[:, kb, :])
                    nc.scalar.dma_start(xtile, xT_d[kb * 128:(kb + 1) * 128, 0:512])
                    xts0.append(xtile)
                # constants follow the x0 stream on the ACT queue; all are
                # first needed at the first rope (~52us) or later. cos2/sin2
                # have mirrored halves (cos duplicated, sin negated), so DMA
                # only the first 64 partitions and reconstruct the rest on
                # the idle DVE — saves 0.5MB of HBM traffic in the
                # bandwidth-critical first ~50us (the sb0->sb1 xta handoff
                # stalls on raw DMA bandwidth).
                nc.scalar.dma_start(cos2[0:64, :], cos2_d[0:64, :])
                nc.scalar.dma_start(sin2[0:64, :], sin2_d[0:64, :])
                nc.scalar.dma_start(swpm, swpm_d[:, :])
                nc.scalar.dma_start(tri, tri_d[:, :])
                nc.scalar.dma_start(onesel, onesel_d[:, :])
                nc.scalar.dma_start(selbc, selbc_d[:, :])
                with nc.allow_low_precision(reason="bf16 copy/negate exact"):
                    nc.vector.tensor_copy(cos2[64:128, :], cos2[0:64, :])
                    nc.vector.tensor_scalar_mul(
                        sin2[64:128, :], sin2[0:64, :], -1.0
                    )
                # xT prefetch for sb>=1 on the SP queue. Only sb=1 is issued
                # here: issuing all three up front makes sb2/sb3's transfers
                # steal DMA bandwidth from sb1's, which stalls the PE at the
                # sb0->sb1 handoff (~52us). sb2/sb3 are issued lazily from
                # the sb loop below, one block ahead of consumption.
                xta = {}

                def prefetch_xta(sb):
                    t = xta_pool.tile([128, KB, 512], bf16, tag="xta",
                                      name=f"xa{sb}")
                    for c in range(4):
                        ksl = slice(c * (KB // 4), (c + 1) * (KB // 4))
                        nc.sync.dma_start(
                            t[:, ksl, :],
                            xv_view[:, ksl, sb * 512:(sb + 1) * 512],
                        )
                    xta[sb] = t

                prefetch_xta(1)

                def rope_or_v(sb, db, ps):
                    """Consume projection PSUM group ps for d-block db."""
                    copy_eng = nc.scalar.copy
                    sl = slice(sb * 512, (sb + 1) * 512)
                    if db == 1:
                        vstg = vs.tile([128, 512], bf16, tag="vstg")
                        copy_eng(vstg, ps)
                        for j in range(4):
                            nc.sync.dma_start_transpose(
                                vts[sb][:, j, :], vstg[:, j * 128:(j + 1) * 128]
                            )
                    else:
                        # RoPE in bf16: swap matmul runs at 1 cyc/col (fp32
                        # would be 4x slower); rotation error ~0.4% is below
                        # the bf16 noise already present in q/k.
                        dst = qk[nq][sb] if db == 0 else qk[db - 2][sb]
                        pcp = rt.tile([128, 512], bf16, tag="pcp")
                        copy_eng(pcp, ps)
                        sps = spp.tile([128, 512], f32, tag="sps")
                        nc.tensor.matmul(sps, swpm, pcp, start=True, stop=True)
                        # sin-mul first: it reads the sps PSUM bank, freeing
                        # it for the next rope's swap matmul immediately.
                        swp = rt.tile([128, 512], bf16, tag="swp")
                        with nc.allow_low_precision(reason="bf16 rope"):
                            nc.vector.tensor_mul(swp, sps, sin2[:, sl])
                            nc.vector.tensor_mul(pcp, pcp, cos2[:, sl])
                            nc.vector.tensor_add(dst[:, :], pcp, swp)

                pend = []  # (sb, db, ps) awaiting rope/v consumption

                # sb=0: kb-outer so PE starts as soon as x0_0/w0 land
                ps0 = [
                    pp.tile([128, 512], f32, tag="pp", name=f"ps0_{db}")
                    for db in range(NDB)
                ]
                for kb in range(KB):
                    for db in range(NDB):
                        nc.tensor.matmul(
                            ps0[db],
                            w_sb[:, kb, db * 128:(db + 1) * 128],
                            xts0[kb],
                            start=(kb == 0),
                            stop=(kb == KB - 1),
                        )
                for db in range(NDB):
                    pend.append((0, db, ps0[db]))

                # sb>=1: db-outer (weights resident, xta prefetched);
                # drain pending epilogues one projection group behind.
                for sb in range(1, SBn):
                    if sb + 1 < SBn:
                        prefetch_xta(sb + 1)
                    for db in range(NDB):
                        ps = pp.tile([128, 512], f32, tag="pp")
                        for kb in range(KB):
                            nc.tensor.matmul(
                                ps,
                                w_sb[:, kb, db * 128:(db + 1) * 128],
                                xta[sb][:, kb, :],
                                start=(kb == 0),
                                stop=(kb == KB - 1),
                            )
                        # drain harder near the end of phase 1 so the ACT
                        # queue is clear of pcp copies when the first
                        # attention exps arrive.
                        cap = 1 if (sb == SBn - 1 and db >= 2) else 3
                        while len(pend) > cap:
                            rope_or_v(*pend.pop(0))
                        pend.append((sb, db, ps))
                while pend:
                    rope_or_v(*pend.pop(0))

            # ---------------- Phase 2: attention ----------------
            with tc.tile_pool(name="wosb", bufs=1) as wo_pool:
                ats = [
                    [
                        wo_pool.tile([128, 512], bf16, tag=f"at{i}_{qb}",
                                     name=f"at{i}_{qb}")
                        for qb in range(SBn)
                    ]
                    for i in range(nq)
                ]
                wo_sb = wo_pool.tile([128, nq, hid], bf16, tag="wosb")
                # SP queue: lands after the xta prefetches, well before
                # phase 3; on the ACT queue it would block the first exps.
                nc.sync.dma_start(
                    wo_sb, wo_d[:, :].rearrange("(t p) c -> p t c", p=128)
                )
                with (
                    tc.tile_pool(name="scp", bufs=3, space="PSUM") as scp,
                    tc.tile_pool(name="atp", bufs=1, space="PSUM") as atp,
                    tc.tile_pool(name="dnp", bufs=1, space="PSUM") as dnp,
                    tc.tile_pool(name="exps", bufs=16) as exps,
                    tc.tile_pool(name="dsm", bufs=2) as dsm,
                    tc.tile_pool(name="atu", bufs=10) as aup,
                    tc.tile_pool(name="ex2p", bufs=3) as ex2p,
                ):
                    # qb=0 is all-diagonal (thin score tiles, least exp
                    # runway) — run it last so its stalls border phase 3,
                    # where the scheduler can pull o_proj matmuls forward.
                    for qb in list(range(1, SBn)) + [0]:
                        accs = [
                            atp.tile([128, 512], f32, tag=f"acc{h}",
                                     name=f"acc{h}")
                            for h in range(nq)
                        ]
                        dn4 = dnp.tile([nq, 512], f32, tag="dn4")
                        nkt = 4 * qb + 4
                        npair = nkt // 2
                        pend2 = []  # (kt, jstart, w, exs) awaiting acc
                        pendn = []  # (pi, ja, wa, ex2s) awaiting dn matmul

                        def flush_av(kt, jstart, w, exs):
                            last = kt == nkt - 1
                            for h in range(nq):
                                nc.tensor.matmul(
                                    accs[h][:, jstart:512],
                                    vts[kt // 4][:, kt % 4, :],
                                    exs[h][:, :w],
                                    start=(kt == 0),
                                    stop=last,
                                )

                        def flush_dn(*pairs):
                            # pair-summed exp tiles halve the PE's denominator
                            # stream; all 4 heads form ONE accumulation group
                            # on dn4 via the one-hot selector blocks. Pairs are
                            # flushed two-at-a-time with h outer so each head's
                            # selector is loaded once per flush.
                            for h in range(nq):
                                for pi, ja, wa, ex2s in pairs:
                                    nc.tensor.matmul(
                                        dn4[0:nq, ja:512],
                                        onesel[:, nq * h:nq * h + nq],
                                        ex2s[h][:, :wa],
                                        start=(pi == 0 and h == 0),
                                        stop=(pi == npair - 1
                                              and h == nq - 1),
                                    )

                        prev = None  # even-kt (kt, jstart, w, exs)
                        for kt in range(nkt):
                            jstart = max(0, 128 * (kt - 4 * qb))
                            w = 512 - jstart
                            kts = qk[nq][kt // 4]
                            exs = []
                            for h in range(nq):
                                sc = scp.tile([128, 512], f32, tag="sc")
                                nc.tensor.matmul(
                                    sc[:, :w],
                                    kts[:, (kt % 4) * 128:(kt % 4 + 1) * 128],
                                    qk[h][qb][:, jstart:512],
                                    start=True,
                                    stop=True,
                                )
                                ex = exps.tile([128, 512], bf16, tag="ex")
                                nc.scalar.activation(ex[:, :w], sc[:, :w], Exp)
                                if kt >= 4 * qb:
                                    nc.vector.tensor_mul(
                                        ex[:, 0:128], ex[:, 0:128], tri
                                    )
                                exs.append(ex)
                            if kt % 2 == 0:
                                prev = (kt, jstart, w, exs)
                            else:
                                ka, ja, wa, exa = prev
                                pi = kt // 2
                                ex2s = []
                                for h in range(nq):
                                    # pair-sum on DVE/GpSimd (bf16, 2x rate);
                                    # ex2 columns 0..wa map to q-cols ja..512.
                                    e2 = ex2p.tile([128, 512], bf16,
                                                   tag=f"ex2_{h}")
                                    eng = nc.vector
                                    with nc.allow_low_precision(
                                        reason="bf16 exp pair-sum"
                                    ):
                                        if ja == jstart:
                                            eng.tensor_add(
                                                e2[:, :wa], exa[h][:, :wa],
                                                exs[h][:, :w],
                                            )
                                        else:
                                            off = jstart - ja
                                            eng.tensor_add(
                                                e2[:, off:wa],
                                                exa[h][:, off:wa],
                                                exs[h][:, :w],
                                            )
                                            eng.tensor_copy(
                                                e2[:, 0:off], exa[h][:, 0:off]
                                            )
                                    ex2s.append(e2)
                                pendn.append((pi, ja, wa, ex2s))
                                if len(pendn) >= 3:
                                    flush_dn(*pendn[:2])
                                    del pendn[:2]
                            if len(pend2) >= 2:
                                flush_av(*pend2.pop(0))
                            pend2.append((kt, jstart, w, exs))
                        for p in pend2:
                            flush_av(*p)
                        for i in range(0, len(pendn), 2):
                            flush_dn(*pendn[i:i + 2])

                        # epilogue: free dn/acc banks early, then normalize.
                        # Everything runs on DVE/Pool/SP so the ACT queue
                        # stays exp-only (PE score matmuls transitively wait
                        # on the in-order ACT stream for PSUM buffer reuse).
                        atus = []
                        for h in range(nq):
                            atu = aup.tile([128, 512], bf16, tag="atu",
                                           name=f"atu{h}")
                            # last qb: no more exps queued, so ACT can help
                            # drain the acc banks phase 3 is waiting on.
                            # qb 0 is processed LAST (stall-aware reorder):
                            # its acc banks gate phase 3's first po groups,
                            # so split the drain across ACT+DVE there.
                            if qb == 0 and h % 2 == 0:
                                nc.scalar.copy(atu, accs[h])
                            else:
                                nc.vector.tensor_copy(atu, accs[h])
                            atus.append(atu)
                        dinv32 = dsm.tile([nq, 512], f32, tag="dinv32")
                        # ~5x faster than reciprocal(); denominators are in
                        # [1, ~2e7] so the approx edge cases can't trigger,
                        # and 51-ULP error is far below the bf16 cast below.
                        # Reads the dn4 PSUM bank directly (saves the staging
                        # copy); the read also frees the bank for the next qb.
                        nc.vector.reciprocal_approx_fast(dinv32, dn4[0:nq, :])
                        dinv4 = dsm.tile([nq, 512], bf16, tag="dinv")
                        with nc.allow_low_precision(
                            reason="softmax denom fine in bf16"
                        ):
                            nc.vector.tensor_copy(dinv4, dinv32)
                            # broadcast row h of dinv4 across partitions with
                            # a K=4 selector matmul (engines cannot read at a
                            # partition offset, so no direct row copy exists).
                            for h in range(nq):
                                # lands on acc_h's bank (freed by the atu
                                # copy); readers clear ~5us before the next
                                # qb's first acc matmul needs the bank.
                                dnb = atp.tile([128, 512], f32, tag=f"acc{h}",
                                               name=f"dnb{h}")
                                nc.tensor.matmul(
                                    dnb,
                                    selbc[:, h * 128:(h + 1) * 128],
                                    dinv4[0:nq, :],
                                    start=True,
                                    stop=True,
                                )
                                nc.vector.tensor_mul(ats[h][qb], atus[h], dnb)

                # ---------------- Phase 3: output projection ----------------
                with (
                    tc.tile_pool(name="outp", bufs=2, space="PSUM") as outp,
                    tc.tile_pool(name="osb", bufs=6) as osb,
                ):
                    # st order follows attention qb completion order
                    for st in [q * 4 + i for q in list(range(1, SBn)) + [0]
                               for i in range(4)]:
                        ssl = slice((st % 4) * 128, (st % 4 + 1) * 128)
                        osl = slice(st * 128, (st + 1) * 128)
                        for nb in range(HB):
                            nsl = slice(nb * 512, (nb + 1) * 512)
                            po = outp.tile([128, 512], f32, tag="po")
                            for h in range(nq):
                                nc.tensor.matmul(
                                    po,
                                    ats[h][st // 4][:, ssl],
                                    wo_sb[:, h, nsl],
                                    start=(h == 0),
                                    stop=(h == nq - 1),
                                )
                            ot = osb.tile([128, 512], bf16, tag="ot")
                            if (st * HB + nb) % 2 == 0:
                                nc.scalar.copy(ot, po)
                            else:
                                nc.vector.tensor_copy(ot, po)
                            nc.sync.dma_start(out_d[osl, nsl], ot)

    nc.compile()
    nc.finalize()
    return nc


def _prep_core_inputs(x_np, position_ids, Wq, Wk, Wv, Wo):
    """Host-side sharding/layout prep. Returns list of per-core input dicts."""
    scale = float(D) ** -0.5
    xT = np.ascontiguousarray(x_np.T).astype(BF)

    pos = np.asarray(position_ids).astype(np.float32)
    half = D // 2
    inv_freq = 1.0 / (ROPE_THETA ** (np.arange(half, dtype=np.float32) / half))
    ang = pos[:, None] * inv_freq[None, :]
    cosT = np.cos(ang).T.astype(np.float32)
    sinT = np.sin(ang).T.astype(np.float32)
    cos2 = np.ascontiguousarray(np.concatenate([cosT, cosT], axis=0)).astype(BF)
    sin2 = np.ascontiguousarray(np.concatenate([-sinT, sinT], axis=0)).astype(BF)

    tri = np.triu(np.ones((128, 128), np.float32)).astype(BF)  # [k, q]: q >= k
    swpm = np.zeros((128, 128), np.float32)
    swpm[np.arange(128), (np.arange(128) + 64) % 128] = 1.0
    swpm = swpm.astype(BF)
    # one-hot selector blocks: head h uses onesel[:, 4h:4h+4] whose column
    # h is all-ones (rest zero), landing its colsum on dn partition h.
    onesel = np.zeros((128, NQ * NQ), np.float32)
    for h in range(NQ):
        onesel[:, NQ * h + h] = 1.0
    # selector for denominator broadcast: block h has row h all-ones, so
    # selbc[:, 128h:128h+128].T @ dinv4 replicates dinv4 row h to all rows.
    selbc = np.zeros((NQ, NQ * 128), np.float32)
    for h in range(NQ):
        selbc[h, h * 128:(h + 1) * 128] = 1.0

    Wq_s = np.asarray(Wq, np.float32) * scale
    Wk = np.asarray(Wk, np.float32)
    Wv = np.asarray(Wv, np.float32)
    Wo = np.asarray(Wo, np.float32)

    in_maps = []
    for c in range(NCORES):
        qrows = Wq_s[c * NQ * D:(c + 1) * NQ * D]
        krows = Wk[c * D:(c + 1) * D]
        vrows = Wv[c * D:(c + 1) * D]
        wqkv = np.concatenate([krows, vrows, qrows], axis=0)
        wqkvT = np.ascontiguousarray(wqkv.T).astype(BF)
        woT = np.ascontiguousarray(
            Wo[:, c * NQ * D:(c + 1) * NQ * D].T
        ).astype(BF)
        in_maps.append(
            {
                "xT": xT,
                "wqkvT": wqkvT,
                "woT": woT,
                "cos2": cos2,
                "sin2": sin2,
                "tri": tri,
                "swpm": swpm,
                "onesel": onesel.astype(BF),
                "selbc": selbc.astype(BF),
            }
        )
    return in_maps


def kernel(
    hidden_states,
    position_ids,
    page_indices,
    Wq,
    Wk,
    Wv,
    Wo,
    kv_cache,
    _trace=False,
):
    from concourse.bass_utils import run_bass_kernel_spmd

    x = np.asarray(hidden_states, np.float32)[0]
    pidx = np.asarray(page_indices)
    assert len(np.unique(pidx)) == pidx.shape[0], "page_indices must be distinct"

    in_maps = _prep_core_inputs(x, position_ids, Wq, Wk, Wv, Wo)

    if "nc" not in _NC_CACHE:
        _NC_CACHE["nc"] = build_nc()
    nc = _NC_CACHE["nc"]

    res = run_bass_kernel_spmd(
        nc, in_maps, core_ids=list(range(NCORES)), trace=_trace,
        trace_cores=list(range(NCORES)) if _trace else None,
    )
    out = np.zeros((S, HID), np.float32)
    for c in range(NCORES):
        out += np.asarray(res.results[c]["out"], np.float32)
    if _trace:
        kernel.last_results = res
    return out[None].astype(np.float32)

